# revision 20
# baseline (speedup 1.0000x reference)
"""GAT layer (nn_GATLayer) as a Bass/Tile SPMD kernel on 8 trn2 NeuronCores.

Row-sharded: core c owns output rows [c*1024, (c+1)*1024).
  h = x @ W                       (local block + AllGather, fp16)
  e = leaky_relu(s_src[i] + s_dst[j]), s_* = h @ a_*
  masked = where(nbr>0, e, 0) == leaky_relu(nbr * (s_src[i]+s_dst[j]))
  att = softmax(masked, axis=1)   (no max-subtraction needed: |z| small)
  out = elu(att @ h)
Softmax denominator comes from a ones-column appended to h in the
aggregation matmul; division + elu applied on the [128,128] result tile.
"""

import sys

for _p in ("/opt/trn_rl_repo",):
    if _p not in sys.path:
        sys.path.insert(0, _p)

import numpy as np

N_CORES = 8
N = 8192               # nodes
D_IN = 512             # input features
D_OUT = 128            # output features
ROWS = N // N_CORES    # rows per core (1024)
N_IT = ROWS // 128     # i-tiles per core (8)
N_JT = N // 128        # j-tiles (64)
HCOL = 132             # h row: 128 features + 1.0 + padding (4B aligned)

# -------- engine assignment knobs (tuned from traces) --------
Z_ENGINE = ["g", "g", "g", "g", "g", "g", "g", "g"]       # z = s_dst + s_src
ZM_ENGINE = ["v", "v", "v", "v", "v", "v", "v", "g"]      # zm = z * mask
LEAKY_ENGINE = ["a", "a", "a", "a", "a", "v", "v", "v"]   # per i-tile: ACT / DVE
CHUNK = 16             # j-subtiles per PSUM staging chunk (16*128 = 2048 cols)
M_BUFS = 4             # mask tile buffering (halves)

_BUILt = {}


def _build_nc():
    import concourse.bacc as bacc
    import concourse.tile as tile
    from concourse import mybir

    f32 = mybir.dt.float32
    f16 = mybir.dt.float16
    i32 = mybir.dt.int32
    AF = mybir.ActivationFunctionType
    OP = mybir.AluOpType

    nc = bacc.Bacc("TRN2", target_bir_lowering=False, debug=False,
                   num_devices=N_CORES)
    import os as _os
    _de = _os.environ.get("GAT_DMA", "scalar")
    DMA = {"scalar": nc.scalar.dma_start, "sync": nc.sync.dma_start,
           "gpsimd": nc.gpsimd.dma_start}[_de]

    x_in = nc.declare_dram_parameter("x_blk", [ROWS, D_IN], f32, isOutput=False)
    nbr_in = nc.declare_dram_parameter("nbr", [ROWS, N], i32, isOutput=False)
    w_in = nc.declare_dram_parameter("w", [D_IN, D_OUT], f32, isOutput=False)
    att_in = nc.declare_dram_parameter("att", [1, 2 * D_OUT], f32, isOutput=False)
    id_in = nc.declare_dram_parameter("ident", [128, 128], f32, isOutput=False)
    out_d = nc.declare_dram_parameter("out", [ROWS, D_OUT], f32, isOutput=True)

    nbr_r = nbr_in[:, :].rearrange("(t p) j -> t p j", p=128)
    out_r = out_d[:, :].rearrange("(t p) n -> t p n", p=128)

    with tile.TileContext(nc) as tc:
        with (
            tc.tile_pool(name="const", bufs=1) as const,
            tc.tile_pool(name="dram", bufs=1, space="DRAM") as dram,
            tc.tile_pool(name="sm", bufs=2) as sm,
            tc.tile_pool(name="mpool", bufs=M_BUFS) as mpool,
            tc.tile_pool(name="zpool", bufs=4) as zpool,
            tc.tile_pool(name="ptpool", bufs=2) as ptpool,
            tc.tile_pool(name="stage_ps", bufs=2, space="PSUM") as stage_ps,
            tc.tile_pool(name="hh_ps", bufs=2, space="PSUM") as hh_ps,
        ):
            # ---------------- constants ----------------
            ident32 = const.tile([128, 128], f32)
            DMA(out=ident32, in_=id_in[:, :])
            ident16 = const.tile([128, 128], f16)
            nc.vector.tensor_copy(out=ident16, in_=ident32)
            att_row = const.tile([1, 2 * D_OUT], f32)
            DMA(out=att_row, in_=att_in[:, :])
            ones_1 = const.tile([1, 128], f32)
            nc.vector.memset(ones_1, 1.0)

            # att broadcast across partitions: [128, 256] via K=1 matmul
            att_bc = const.tile([128, 2 * D_OUT], f32)
            s_src_sb = const.tile([128, N_IT], f32)
            s_dst_sb = const.tile([128, N_IT], f32)
            sdb = const.tile([128, N], f16)          # s_dst broadcast, j-major
            h_aug = const.tile([128, N_JT, HCOL], f16)  # [j', jt, 128 feats + 1.0]

            with (
                tc.tile_pool(name="pre_sb", bufs=1) as pre_sb,
                tc.tile_pool(name="pre_ps", bufs=2, space="PSUM") as pre_ps,
            ):
                att_ps = pre_ps.tile([128, 2 * D_OUT], f32, tag="pp")
                nc.tensor.matmul(out=att_ps, lhsT=ones_1, rhs=att_row,
                                 start=True, stop=True)
                nc.scalar.copy(out=att_bc, in_=att_ps)

                # x block + W
                x_sb = pre_sb.tile([128, N_IT, D_IN], f32)
                DMA(
                    out=x_sb, in_=x_in[:, :].rearrange("(s p) d -> p s d", p=128))
                w_sb = pre_sb.tile([128, 4, D_OUT], f32)
                DMA(
                    out=w_sb, in_=w_in[:, :].rearrange("(t p) n -> p t n", p=128))

                # transpose x: xt[d', t, s, i'] = x[s*128+i', t*128+d']
                xt_sb = pre_sb.tile([128, 4, N_IT, 128], f32)
                for s in range(N_IT):
                    for t in range(4):
                        xt_ps = pre_ps.tile([128, 128], f32, tag="pp")
                        nc.tensor.transpose(
                            out=xt_ps, in_=x_sb[:, s, t * 128:(t + 1) * 128],
                            identity=ident32)
                        nc.scalar.copy(out=xt_sb[:, t, s, :], in_=xt_ps)

                # h_local per i-subtile + attention dots
                h16_sb = pre_sb.tile([128, N_IT, HCOL], f16)
                nc.vector.memset(h16_sb[:, :, D_OUT:], 0.0)
                nc.gpsimd.memset(h16_sb[:, :, D_OUT:D_OUT + 1], 1.0)
                scrap = pre_sb.tile([128, 128], f32)
                scrap2 = pre_sb.tile([128, 128], f32)
                for s in range(N_IT):
                    h_ps = pre_ps.tile([128, D_OUT], f32, tag="pp")
                    for t in range(4):
                        nc.tensor.matmul(out=h_ps, lhsT=xt_sb[:, t, s, :],
                                         rhs=w_sb[:, t, :],
                                         start=(t == 0), stop=(t == 3))
                    nc.vector.tensor_mul(scrap, h_ps, att_bc[:, :D_OUT])
                    nc.vector.tensor_reduce(
                        out=s_src_sb[:, s:s + 1], in_=scrap,
                        axis=mybir.AxisListType.X, op=OP.add)
                    nc.vector.tensor_mul(scrap2, h_ps, att_bc[:, D_OUT:])
                    nc.vector.tensor_reduce(
                        out=s_dst_sb[:, s:s + 1], in_=scrap2,
                        axis=mybir.AxisListType.X, op=OP.add)
                    nc.scalar.copy(out=h16_sb[:, s, :D_OUT], in_=h_ps)

                # s_dst -> [8, 128] (j-ordered) fp16 for the gather
                sdt_ps = pre_ps.tile([N_IT, 128], f32, tag="pp")
                nc.tensor.transpose(out=sdt_ps, in_=s_dst_sb, identity=ident32)
                sdt16 = pre_sb.tile([N_IT, 128], f16)
                nc.vector.tensor_copy(out=sdt16, in_=sdt_ps)

                # ---------------- collectives ----------------
                _stop0 = _os.environ.get("GAT_STOP", "full")
                h16_loc = dram.tile([ROWS, HCOL], f16)
                h16_full = dram.tile([N, HCOL], f16)
                sd_loc = dram.tile([N_IT, 128], f16)
                sd_full = dram.tile([N_CORES * N_IT, 128], f16)
                if _stop0 != "pre0":
                    DMA(
                        out=h16_loc[:, :].rearrange("(s p) c -> p s c", p=128),
                        in_=h16_sb)
                    DMA(out=sd_loc, in_=sdt16)
                    if _os.environ.get("GAT_NO_COLLECTIVE"):
                        DMA(out=h16_full[:ROWS, :], in_=h16_loc[:, :])
                        DMA(out=sd_full[:N_IT, :], in_=sd_loc[:, :])
                    else:
                        nc.gpsimd.collective_compute(
                            "AllGather", OP.bypass,
                            replica_groups=[list(range(N_CORES))],
                            ins=[h16_loc[:, :].opt()], outs=[h16_full[:, :].opt()])
                        nc.gpsimd.collective_compute(
                            "AllGather", OP.bypass,
                            replica_groups=[list(range(N_CORES))],
                            ins=[sd_loc[:, :].opt()], outs=[sd_full[:, :].opt()])

                    DMA(
                        out=h_aug,
                        in_=h16_full[:, :].rearrange("(t p) c -> p t c", p=128))
                    # broadcast s_dst to all partitions (partition-step-0 AP)
                    sd_flat = sd_full[:, :]
                    import concourse.bass as bass
                    sd_bcast_ap = bass.AP(
                        tensor=sd_flat.tensor, offset=sd_flat.offset,
                        ap=[[0, 128], [1, N]])
                    nc.gpsimd.dma_start(out=sdb, in_=sd_bcast_ap)

            # ---------------- main loop over i-tiles ----------------
            _stop = _os.environ.get("GAT_STOP", "full")
            HALF = N // 2
            if _stop in ("pre", "pre0"):
                for it in range(N_IT):
                    o_t = sm.tile([128, D_OUT], f32, tag="ot")
                    nc.vector.tensor_scalar_mul(o_t, att_bc[:, :D_OUT], 1.0)
                    DMA(out=out_r[it], in_=o_t)
            for it in range(N_IT if _stop not in ("pre", "pre0") else 0):
                halves = []
                for hf in range(2):
                    sl = slice(hf * HALF, (hf + 1) * HALF)
                    m_t = mpool.tile([128, HALF], i32, tag="m")
                    DMA(out=m_t, in_=nbr_r[it, :, sl])
                    z_t = zpool.tile([128, HALF], f16, tag="z")
                    if ZM_ENGINE[it] == "v":
                        # fused: zm = (s_dst + s_src) * mask, one DVE op
                        nc.vector.scalar_tensor_tensor(
                            out=z_t, in0=sdb[:, sl],
                            scalar=s_src_sb[:, it:it + 1], in1=m_t,
                            op0=OP.add, op1=OP.mult)
                    else:
                        # gpsimd lacks TensorScalarPtr: two-op fallback
                        nc.gpsimd.tensor_scalar_add(
                            z_t, sdb[:, sl], s_src_sb[:, it:it + 1])
                        nc.gpsimd.tensor_tensor(
                            out=z_t, in0=z_t, in1=m_t, op=OP.mult)
                    if LEAKY_ENGINE[it] == "a":
                        nc.scalar.activation(
                            out=z_t, in_=z_t, func=AF.Prelu, alpha=0.2)
                    else:
                        nc.vector.scalar_tensor_tensor(
                            out=z_t, in0=z_t,
                            scalar=0.2, in1=z_t, op0=OP.mult, op1=OP.max)
                    halves.append(z_t)

                if _stop == "zm":
                    o_t = sm.tile([128, D_OUT], f32, tag="ot")
                    nc.vector.tensor_copy(out=o_t, in_=halves[0][:, :D_OUT])
                    DMA(out=out_r[it], in_=o_t)
                    continue
                pT = ptpool.tile([128, N], f16)
                hh = hh_ps.tile([128, D_OUT + 1], f32, tag="hh")
                for g in range(N_JT // CHUNK):
                    stage = stage_ps.tile([128, CHUNK * 128], f16, tag="stage")
                    for jj in range(CHUNK):
                        jt = g * CHUNK + jj
                        src = halves[jt // 32]
                        jo = jt % 32
                        nc.tensor.transpose(
                            out=stage[:, jj * 128:(jj + 1) * 128],
                            in_=src[:, jo * 128:(jo + 1) * 128],
                            identity=ident16)
                    nc.scalar.activation(
                        out=pT[:, g * CHUNK * 128:(g + 1) * CHUNK * 128],
                        in_=stage, func=AF.Exp)
                    for jj in range(CHUNK):
                        jt = g * CHUNK + jj
                        nc.tensor.matmul(
                            out=hh, lhsT=pT[:, jt * 128:(jt + 1) * 128],
                            rhs=h_aug[:, jt, :D_OUT + 1],
                            start=(jt == 0), stop=(jt == N_JT - 1))

                if _stop == "tr":
                    o_t = sm.tile([128, D_OUT], f32, tag="ot")
                    nc.vector.tensor_copy(out=o_t, in_=pT[:, :D_OUT])
                    DMA(out=out_r[it], in_=o_t)
                    continue
                # out = elu(hh[:, :128] / Z),  Z = hh[:, 128]
                rz = sm.tile([128, 1], f32, tag="rz")
                nc.vector.reciprocal(out=rz, in_=hh[:, D_OUT:D_OUT + 1])
                tmin = sm.tile([128, D_OUT], f32, tag="tmin")
                nc.vector.tensor_scalar_min(tmin, hh[:, :D_OUT], 0.0)
                wmax = sm.tile([128, D_OUT], f32, tag="wmax")
                nc.vector.tensor_scalar(
                    out=wmax, in0=hh[:, :D_OUT], scalar1=0.0, scalar2=rz,
                    op0=OP.max, op1=OP.mult)
                e_t = sm.tile([128, D_OUT], f32, tag="et")
                nc.scalar.activation(out=e_t, in_=tmin, func=AF.Exp, scale=rz)
                o_t = sm.tile([128, D_OUT], f32, tag="ot")
                nc.vector.scalar_tensor_tensor(
                    out=o_t, in0=e_t, scalar=-1.0, in1=wmax,
                    op0=OP.add, op1=OP.add)
                DMA(out=out_r[it], in_=o_t)

    nc.compile()
    return nc


def _get_nc():
    if "nc" not in _BUILt:
        _BUILt["nc"] = _build_nc()
    return _BUILt["nc"]


_last_exec_ns = None


def kernel(x, immediate_neighbor, weights, attention):
    import os
    from concourse.bass_utils import run_bass_kernel_spmd

    x = np.asarray(x, dtype=np.float32)
    nbr = np.asarray(immediate_neighbor, dtype=np.int32)
    w = np.asarray(weights, dtype=np.float32)
    att = np.asarray(attention, dtype=np.float32).reshape(1, 2 * D_OUT)
    ident = np.eye(128, dtype=np.float32)

    nc = _get_nc()
    in_maps = []
    for c in range(N_CORES):
        in_maps.append({
            "x_blk": x[c * ROWS:(c + 1) * ROWS],
            "nbr": nbr[c * ROWS:(c + 1) * ROWS],
            "w": w,
            "att": att,
            "ident": ident,
        })
    kw = {}
    if os.environ.get("GAT_TRACE"):
        kw["trace"] = True
        tdir = os.environ.get("GAT_TRACE_DIR", "/tmp/gat_trace")
        os.makedirs(tdir, exist_ok=True)
        kw["tmpdir"] = tdir
    res = run_bass_kernel_spmd(nc, in_maps, list(range(N_CORES)), **kw)
    global _last_exec_ns
    _last_exec_ns = res.exec_time_ns
    out = np.concatenate([res.results[c]["out"] for c in range(N_CORES)], axis=0)
    return out.astype(np.float32)


# revision 25
# speedup vs baseline: 1.1658x; 1.1658x over previous
"""GAT layer (nn_GATLayer) as a Bass/Tile SPMD kernel on 8 trn2 NeuronCores.

Row-sharded: core c owns output rows [c*1024, (c+1)*1024).
  h = x @ W                       (local block + AllGather, fp16)
  e = leaky_relu(s_src[i] + s_dst[j]), s_* = h @ a_*
  masked = where(nbr>0, e, 0) == leaky_relu(nbr * (s_src[i]+s_dst[j]))
  att = softmax(masked, axis=1)   (no max-subtraction needed: |z| small)
  out = elu(att @ h)
Softmax denominator comes from a ones-column appended to h in the
aggregation matmul; division + elu applied on the [128,128] result tile.
"""

import sys

for _p in ("/opt/trn_rl_repo",):
    if _p not in sys.path:
        sys.path.insert(0, _p)

import numpy as np

N_CORES = 8
N = 8192               # nodes
D_IN = 512             # input features
D_OUT = 128            # output features
ROWS = N // N_CORES    # rows per core (1024)
N_IT = ROWS // 128     # i-tiles per core (8)
N_JT = N // 128        # j-tiles (64)
HCOL = 132             # h row: 128 features + 1.0 + padding (4B aligned)

# -------- engine assignment knobs (tuned from traces) --------
Z_ENGINE = ["g", "g", "g", "g", "g", "g", "g", "g"]       # z = s_dst + s_src
ZM_ENGINE = ["v", "v", "v", "v", "v", "v", "v", "v"]      # zm = z * mask
LEAKY_ENGINE = ["a", "a", "a", "a", "a", "v", "v", "v"]   # per i-tile: ACT / DVE
CHUNK = 16             # j-subtiles per PSUM staging chunk (16*128 = 2048 cols)
M_BUFS = 4             # mask tile buffering (halves)

_BUILt = {}


def _build_nc():
    import concourse.bacc as bacc
    import concourse.tile as tile
    from concourse import mybir

    f32 = mybir.dt.float32
    f16 = mybir.dt.float16
    i32 = mybir.dt.int32
    AF = mybir.ActivationFunctionType
    OP = mybir.AluOpType

    nc = bacc.Bacc("TRN2", target_bir_lowering=False, debug=False,
                   num_devices=N_CORES)
    import os as _os
    _de = _os.environ.get("GAT_DMA", "scalar")
    DMA = {"scalar": nc.scalar.dma_start, "sync": nc.sync.dma_start,
           "gpsimd": nc.gpsimd.dma_start}[_de]

    x_in = nc.declare_dram_parameter("x_blk", [ROWS, D_IN], f32, isOutput=False)
    nbr_in = nc.declare_dram_parameter("nbr", [ROWS, N], i32, isOutput=False)
    w_in = nc.declare_dram_parameter("w", [D_IN, D_OUT], f32, isOutput=False)
    att_in = nc.declare_dram_parameter("att", [1, 2 * D_OUT], f32, isOutput=False)
    id_in = nc.declare_dram_parameter("ident", [128, 128], f32, isOutput=False)
    out_d = nc.declare_dram_parameter("out", [ROWS, D_OUT], f32, isOutput=True)

    nbr_r = nbr_in[:, :].rearrange("(t p) j -> t p j", p=128)
    out_r = out_d[:, :].rearrange("(t p) n -> t p n", p=128)

    with tile.TileContext(nc) as tc:
        with (
            tc.tile_pool(name="const", bufs=1) as const,
            tc.tile_pool(name="dram", bufs=1, space="DRAM") as dram,
            tc.tile_pool(name="sm", bufs=2) as sm,
            tc.tile_pool(name="mpool", bufs=M_BUFS) as mpool,
            tc.tile_pool(name="zpool", bufs=4) as zpool,
            tc.tile_pool(name="ptpool", bufs=2) as ptpool,
            tc.tile_pool(name="stage_ps", bufs=2, space="PSUM") as stage_ps,
            tc.tile_pool(name="hh_ps", bufs=2, space="PSUM") as hh_ps,
        ):
            # ---------------- constants ----------------
            ident32 = const.tile([128, 128], f32)
            DMA(out=ident32, in_=id_in[:, :])
            ident16 = const.tile([128, 128], f16)
            nc.vector.tensor_copy(out=ident16, in_=ident32)
            att_row = const.tile([1, 2 * D_OUT], f32)
            DMA(out=att_row, in_=att_in[:, :])
            ones_1 = const.tile([1, 128], f32)
            nc.vector.memset(ones_1, 1.0)

            # att broadcast across partitions: [128, 256] via K=1 matmul
            att_bc = const.tile([128, 2 * D_OUT], f32)
            s_src_sb = const.tile([128, N_IT], f32)
            s_dst_sb = const.tile([128, N_IT], f32)
            sdb = const.tile([128, N], f16)          # s_dst broadcast, j-major
            h_aug = const.tile([128, N_JT, HCOL], f16)  # [j', jt, 128 feats + 1.0]

            with (
                tc.tile_pool(name="pre_sb", bufs=1) as pre_sb,
                tc.tile_pool(name="pre_ps", bufs=2, space="PSUM") as pre_ps,
            ):
                att_ps = pre_ps.tile([128, 2 * D_OUT], f32, tag="pp")
                nc.tensor.matmul(out=att_ps, lhsT=ones_1, rhs=att_row,
                                 start=True, stop=True)
                nc.scalar.copy(out=att_bc, in_=att_ps)

                # x block + W
                x_sb = pre_sb.tile([128, N_IT, D_IN], f32)
                DMA(
                    out=x_sb, in_=x_in[:, :].rearrange("(s p) d -> p s d", p=128))
                w_sb = pre_sb.tile([128, 4, D_OUT], f32)
                DMA(
                    out=w_sb, in_=w_in[:, :].rearrange("(t p) n -> p t n", p=128))

                # transpose x: xt[d', t, s, i'] = x[s*128+i', t*128+d']
                xt_sb = pre_sb.tile([128, 4, N_IT, 128], f32)
                for s in range(N_IT):
                    for t in range(4):
                        xt_ps = pre_ps.tile([128, 128], f32, tag="pp")
                        nc.tensor.transpose(
                            out=xt_ps, in_=x_sb[:, s, t * 128:(t + 1) * 128],
                            identity=ident32)
                        nc.scalar.copy(out=xt_sb[:, t, s, :], in_=xt_ps)

                # h_local per i-subtile + attention dots
                h16_sb = pre_sb.tile([128, N_IT, HCOL], f16)
                nc.vector.memset(h16_sb[:, :, D_OUT:], 0.0)
                nc.gpsimd.memset(h16_sb[:, :, D_OUT:D_OUT + 1], 1.0)
                scrap = pre_sb.tile([128, 128], f32)
                scrap2 = pre_sb.tile([128, 128], f32)
                for s in range(N_IT):
                    h_ps = pre_ps.tile([128, D_OUT], f32, tag="pp")
                    for t in range(4):
                        nc.tensor.matmul(out=h_ps, lhsT=xt_sb[:, t, s, :],
                                         rhs=w_sb[:, t, :],
                                         start=(t == 0), stop=(t == 3))
                    nc.vector.tensor_mul(scrap, h_ps, att_bc[:, :D_OUT])
                    nc.vector.tensor_reduce(
                        out=s_src_sb[:, s:s + 1], in_=scrap,
                        axis=mybir.AxisListType.X, op=OP.add)
                    nc.vector.tensor_mul(scrap2, h_ps, att_bc[:, D_OUT:])
                    nc.vector.tensor_reduce(
                        out=s_dst_sb[:, s:s + 1], in_=scrap2,
                        axis=mybir.AxisListType.X, op=OP.add)
                    nc.scalar.copy(out=h16_sb[:, s, :D_OUT], in_=h_ps)

                # s_dst -> [8, 128] (j-ordered) fp16 for the gather
                sdt_ps = pre_ps.tile([N_IT, 128], f32, tag="pp")
                nc.tensor.transpose(out=sdt_ps, in_=s_dst_sb, identity=ident32)
                sdt16 = pre_sb.tile([N_IT, 128], f16)
                nc.vector.tensor_copy(out=sdt16, in_=sdt_ps)

                # ---------------- collectives ----------------
                _stop0 = _os.environ.get("GAT_STOP", "full")
                h16_loc = dram.tile([ROWS, HCOL], f16)
                h16_full = dram.tile([N, HCOL], f16)
                sd_loc = dram.tile([N_IT, 128], f16)
                sd_full = dram.tile([N_CORES * N_IT, 128], f16)
                if _stop0 != "pre0":
                    DMA(
                        out=h16_loc[:, :].rearrange("(s p) c -> p s c", p=128),
                        in_=h16_sb)
                    DMA(out=sd_loc, in_=sdt16)
                    if _os.environ.get("GAT_NO_COLLECTIVE"):
                        DMA(out=h16_full[:ROWS, :], in_=h16_loc[:, :])
                        DMA(out=sd_full[:N_IT, :], in_=sd_loc[:, :])
                    else:
                        nc.gpsimd.collective_compute(
                            "AllGather", OP.bypass,
                            replica_groups=[list(range(N_CORES))],
                            ins=[h16_loc[:, :].opt()], outs=[h16_full[:, :].opt()])
                        nc.gpsimd.collective_compute(
                            "AllGather", OP.bypass,
                            replica_groups=[list(range(N_CORES))],
                            ins=[sd_loc[:, :].opt()], outs=[sd_full[:, :].opt()])

                    DMA(
                        out=h_aug,
                        in_=h16_full[:, :].rearrange("(t p) c -> p t c", p=128))
                    # broadcast s_dst to all partitions (partition-step-0 AP)
                    sd_flat = sd_full[:, :]
                    import concourse.bass as bass
                    sd_bcast_ap = bass.AP(
                        tensor=sd_flat.tensor, offset=sd_flat.offset,
                        ap=[[0, 128], [1, N]])
                    nc.gpsimd.dma_start(out=sdb, in_=sd_bcast_ap)

            # ---------------- main loop over i-tiles ----------------
            _stop = _os.environ.get("GAT_STOP", "full")
            HALF = N // 2
            if _stop in ("pre", "pre0"):
                for it in range(N_IT):
                    o_t = sm.tile([128, D_OUT], f32, tag="ot")
                    nc.vector.tensor_scalar_mul(o_t, att_bc[:, :D_OUT], 1.0)
                    DMA(out=out_r[it], in_=o_t)
            for it in range(N_IT if _stop not in ("pre", "pre0") else 0):
                halves = []
                for hf in range(2):
                    sl = slice(hf * HALF, (hf + 1) * HALF)
                    m_t = mpool.tile([128, HALF], i32, tag="m")
                    DMA(out=m_t, in_=nbr_r[it, :, sl])
                    z_t = zpool.tile([128, HALF], f16, tag="z")
                    if ZM_ENGINE[it] == "v":
                        # fused: zm = (s_dst + s_src) * mask, one DVE op
                        nc.vector.scalar_tensor_tensor(
                            out=z_t, in0=sdb[:, sl],
                            scalar=s_src_sb[:, it:it + 1], in1=m_t,
                            op0=OP.add, op1=OP.mult)
                    else:
                        # gpsimd lacks TensorScalarPtr: two-op fallback
                        nc.gpsimd.tensor_scalar_add(
                            z_t, sdb[:, sl], s_src_sb[:, it:it + 1])
                        nc.gpsimd.tensor_tensor(
                            out=z_t, in0=z_t, in1=m_t, op=OP.mult)
                    if LEAKY_ENGINE[it] == "a":
                        nc.scalar.activation(
                            out=z_t, in_=z_t, func=AF.Prelu, alpha=0.2)
                    else:
                        nc.vector.scalar_tensor_tensor(
                            out=z_t, in0=z_t,
                            scalar=0.2, in1=z_t, op0=OP.mult, op1=OP.max)
                    halves.append(z_t)

                if _stop == "zm":
                    o_t = sm.tile([128, D_OUT], f32, tag="ot")
                    nc.vector.tensor_copy(out=o_t, in_=halves[0][:, :D_OUT])
                    DMA(out=out_r[it], in_=o_t)
                    continue
                pT = ptpool.tile([128, N], f16)
                hh = hh_ps.tile([128, D_OUT + 1], f32, tag="hh")
                for g in range(N_JT // CHUNK):
                    stage = stage_ps.tile([128, CHUNK * 128], f16, tag="stage")
                    for jj in range(CHUNK):
                        jt = g * CHUNK + jj
                        src = halves[jt // 32]
                        jo = jt % 32
                        nc.tensor.transpose(
                            out=stage[:, jj * 128:(jj + 1) * 128],
                            in_=src[:, jo * 128:(jo + 1) * 128],
                            identity=ident16)
                    nc.scalar.activation(
                        out=pT[:, g * CHUNK * 128:(g + 1) * CHUNK * 128],
                        in_=stage, func=AF.Exp)
                    for jj in range(CHUNK):
                        jt = g * CHUNK + jj
                        nc.tensor.matmul(
                            out=hh, lhsT=pT[:, jt * 128:(jt + 1) * 128],
                            rhs=h_aug[:, jt, :D_OUT + 1],
                            start=(jt == 0), stop=(jt == N_JT - 1))

                if _stop == "tr":
                    o_t = sm.tile([128, D_OUT], f32, tag="ot")
                    nc.vector.tensor_copy(out=o_t, in_=pT[:, :D_OUT])
                    DMA(out=out_r[it], in_=o_t)
                    continue
                # out = elu(hh[:, :128] / Z),  Z = hh[:, 128]
                rz = sm.tile([128, 1], f32, tag="rz")
                nc.vector.reciprocal(out=rz, in_=hh[:, D_OUT:D_OUT + 1])
                tmin = sm.tile([128, D_OUT], f32, tag="tmin")
                nc.vector.tensor_scalar_min(tmin, hh[:, :D_OUT], 0.0)
                wmax = sm.tile([128, D_OUT], f32, tag="wmax")
                nc.vector.tensor_scalar(
                    out=wmax, in0=hh[:, :D_OUT], scalar1=0.0, scalar2=rz,
                    op0=OP.max, op1=OP.mult)
                e_t = sm.tile([128, D_OUT], f32, tag="et")
                nc.scalar.activation(out=e_t, in_=tmin, func=AF.Exp, scale=rz)
                o_t = sm.tile([128, D_OUT], f32, tag="ot")
                nc.vector.scalar_tensor_tensor(
                    out=o_t, in0=e_t, scalar=-1.0, in1=wmax,
                    op0=OP.add, op1=OP.add)
                DMA(out=out_r[it], in_=o_t)

    nc.compile()
    return nc


def _get_nc():
    if "nc" not in _BUILt:
        _BUILt["nc"] = _build_nc()
    return _BUILt["nc"]


_last_exec_ns = None


def kernel(x, immediate_neighbor, weights, attention):
    import os
    from concourse.bass_utils import run_bass_kernel_spmd

    x = np.asarray(x, dtype=np.float32)
    nbr = np.asarray(immediate_neighbor, dtype=np.int32)
    w = np.asarray(weights, dtype=np.float32)
    att = np.asarray(attention, dtype=np.float32).reshape(1, 2 * D_OUT)
    ident = np.eye(128, dtype=np.float32)

    nc = _get_nc()
    in_maps = []
    for c in range(N_CORES):
        in_maps.append({
            "x_blk": x[c * ROWS:(c + 1) * ROWS],
            "nbr": nbr[c * ROWS:(c + 1) * ROWS],
            "w": w,
            "att": att,
            "ident": ident,
        })
    kw = {}
    if os.environ.get("GAT_TRACE"):
        kw["trace"] = True
        tdir = os.environ.get("GAT_TRACE_DIR", "/tmp/gat_trace")
        os.makedirs(tdir, exist_ok=True)
        kw["tmpdir"] = tdir
    res = run_bass_kernel_spmd(nc, in_maps, list(range(N_CORES)), **kw)
    global _last_exec_ns
    _last_exec_ns = res.exec_time_ns
    out = np.concatenate([res.results[c]["out"] for c in range(N_CORES)], axis=0)
    return out.astype(np.float32)
